# revision 39
# baseline (speedup 1.0000x reference)
"""Causal multi-head attention (dense transformer block) on 8 Trainium2 NeuronCores.

Problem: x[4, 2048, 1024] -> qkv proj (16 heads x 64) -> causal softmax
attention -> out proj W_out + b_out.

Sharding (hardcoded): data-parallel over the 4 batch elements x
tensor-parallel over 2 head groups (8 heads each) = 8 cores. Each core
computes, for its (batch, head-group):
    Q^T, K^T (transposed layout, scale folded into W_q), V
    per 512-query block: scores S^T = K_h^T.T @ Q_h^T  [keys, queries],
    E = exp(S^T) with on-device causal masking, then unnormalized O^T and the softmax
    denominator via one accumulated matmul with a ones-augmented V
    (lhsT = [V_j | 1]), then O^T = O~^T * bcast(1/denominator),
    and a partial output projection with its row-shard of W_out.
The host sums the two partials per batch element and adds b_out.

Diagonal 128-key chunks only stream query columns >= r*128 (the rest are
fully causal-masked): the S matmul, exp, and O matmul are all trimmed to
that range, and the O matmul simply never touches the masked columns
(no zero-fill memsets needed).

Self-contained: hardcodes all shapes; builds/compiles the Bass program on
first call and runs it SPMD on cores 0-7 via run_bass_kernel_spmd.
"""

import numpy as np

B, N, DIM = 4, 2048, 1024
HEADS, DH = 16, 64
HPC = HEADS // 2          # heads per core = 8
INNER = HPC * DH          # per-core inner width = 512
QT = 512                  # query-block tokens
KC = 128                  # key chunk
NQB = N // QT             # 4 query blocks
KD = DIM // 128           # 8 contraction chunks over model dim
IC = INNER // 128         # 4 chunks over per-core inner dim

_cache = {}


def _build_program():
    import concourse.bacc as bacc
    import concourse.mybir as mybir
    import concourse.tile as tile
    from concourse import library_config

    fp32 = mybir.dt.float32
    bf16 = mybir.dt.bfloat16
    Exp = mybir.ActivationFunctionType.Exp

    nc = bacc.Bacc("TRN2", target_bir_lowering=False, debug=False)
    xT = nc.dram_tensor("xT", [DIM, N], bf16, kind="ExternalInput").ap()
    wq = nc.dram_tensor("wq", [DIM, INNER], bf16, kind="ExternalInput").ap()
    wk = nc.dram_tensor("wk", [DIM, INNER], bf16, kind="ExternalInput").ap()
    wv = nc.dram_tensor("wv", [DIM, INNER], bf16, kind="ExternalInput").ap()
    wo = nc.dram_tensor("wo", [INNER, DIM], bf16, kind="ExternalInput").ap()
    out = nc.dram_tensor("out", [N, DIM], fp32, kind="ExternalOutput").ap()

    with tile.TileContext(nc) as tc:
        with (
            tc.tile_pool(name="persist", bufs=1) as pp,
            tc.tile_pool(name="weights", bufs=1) as wp,
            tc.tile_pool(name="xstream", bufs=14) as xp,
            tc.tile_pool(name="otpool", bufs=3) as otp,
            tc.tile_pool(name="epool", bufs=6) as ep,
            tc.tile_pool(name="rcpool", bufs=3) as rcp,
            tc.tile_pool(name="outstage", bufs=3) as osp,
            tc.tile_pool(name="mmpsum", bufs=2, space="PSUM") as mmp,
            tc.tile_pool(name="stpsum", bufs=2, space="PSUM") as stp,
            tc.tile_pool(name="opsum", bufs=2, space="PSUM") as opp,
        ):
            NB = N // QT      # token blocks
            RB = QT // KC     # key chunks per block
            # Kt/Vaug/Qt split per 512-token block so attention on block qi
            # only waits for blocks <= qi (tile-granularity deps)
            Kt = [pp.tile([128, IC, QT], bf16, name=f"Kt_{b}") for b in range(NB)]
            Vaug = [
                pp.tile([128, RB, HPC, DH + 1], bf16, name=f"Vaug_{b}")
                for b in range(NB)
            ]
            Qt = [pp.tile([128, IC, QT], bf16, name=f"Qt_{b}") for b in range(NB)]
            # per-chunk weight tiles (tile-granularity deps: the first
            # matmul only waits for its own 128-row chunk, not the whole
            # weight matrix)
            wq_sb = [wp.tile([128, INNER], bf16, name=f"wq_sb_{k}") for k in range(KD)]
            wk_sb = [wp.tile([128, INNER], bf16, name=f"wk_sb_{k}") for k in range(KD)]
            wv_sb = [wp.tile([128, INNER], bf16, name=f"wv_sb_{k}") for k in range(KD)]
            wo_sb = wp.tile([128, IC, DIM], bf16, name="wo_sb")

            # preload the gpsimd custom-op library that partition_broadcast
            # lives in, so the UNLOAD/LOAD swap (~7us, stalls every engine
            # behind it) happens during the startup DMA wait instead of in
            # the middle of the first attention epilogue
            nc.gpsimd.load_library(library_config.attn)

            xas = {}

            def emit_x_loads(ni):
                xa = []
                for k in range(KD):
                    t = xp.tile([128, QT], bf16, tag="x", name=f"xa_{ni}_{k}")
                    nc.sync.dma_start(
                        t[:], xT[k * 128 : (k + 1) * 128, ni * QT : (ni + 1) * QT]
                    )
                    xa.append(t)
                xas[ni] = xa

            # interleave the startup loads so the first K-projection matmul
            # is ready after two small DMAs instead of the full weight set
            xa0 = []
            for k in range(KD):
                nc.sync.dma_start(wk_sb[k][:], wk[k * 128 : (k + 1) * 128, :])
                t = xp.tile([128, QT], bf16, tag="x", name=f"xa_0_{k}")
                nc.sync.dma_start(t[:], xT[k * 128 : (k + 1) * 128, 0:QT])
                xa0.append(t)
            xas[0] = xa0
            # wq before wv: the first attention block needs K and Q to start
            # its S/exp pipeline, while V is only consumed by the O matmuls
            # (which trail by a pipeline step) — starts attention ~2us sooner
            for k in range(KD):
                nc.sync.dma_start(wq_sb[k][:], wq[k * 128 : (k + 1) * 128, :])
            for k in range(KD):
                nc.sync.dma_start(wv_sb[k][:], wv[k * 128 : (k + 1) * 128, :])
            nc.sync.dma_start(wo_sb[:], wo.rearrange("(ko p) m -> p ko m", p=128))
            for b in range(NB):
                nc.vector.memset(Vaug[b][:, :, :, DH], 1.0)

            def proj_gen(ni, kinds):
                """Generator emitting projection matmuls one at a time so the
                stage driver can pump single matmuls into the PE gaps of
                ACT-paced attention stretches. kinds is a string of
                'k'/'v'/'q' selecting which projections to emit."""
                xa = xas[ni]
                for kind in kinds:
                    if kind == "k":
                        for c in range(IC):
                            ps = mmp.tile([128, QT], fp32, tag="mm", name=f"kps_{ni}_{c}")
                            for k in range(KD):
                                nc.tensor.matmul(
                                    ps[:],
                                    wk_sb[k][:, c * 128 : (c + 1) * 128],
                                    xa[k][:],
                                    start=(k == 0),
                                    stop=(k == KD - 1),
                                )
                                if k < KD - 1:
                                    yield
                            nc.vector.tensor_copy(Kt[ni][:, c, :], ps[:])
                            yield
                    elif kind == "v":
                        for t_ in range(RB):
                            ps = mmp.tile([128, INNER], fp32, tag="mm", name=f"vps_{ni}_{t_}")
                            for k in range(KD):
                                nc.tensor.matmul(
                                    ps[:],
                                    xa[k][:, t_ * KC : (t_ + 1) * KC],
                                    wv_sb[k][:],
                                    start=(k == 0),
                                    stop=(k == KD - 1),
                                )
                                if k < KD - 1:
                                    yield
                            # ACT copy: keeps the DVE queue short so PE's
                            # attnv LDWEIGHTS isn't stuck behind it
                            nc.scalar.copy(
                                Vaug[ni][:, t_, :, 0:DH],
                                ps[:].rearrange("p (h d) -> p h d", h=HPC),
                            )
                            yield
                    else:
                        for c in range(IC):
                            ps = mmp.tile([128, QT], fp32, tag="mm", name=f"qps_{ni}_{c}")
                            for k in range(KD):
                                nc.tensor.matmul(
                                    ps[:],
                                    wq_sb[k][:, c * 128 : (c + 1) * 128],
                                    xa[k][:],
                                    start=(k == 0),
                                    stop=(k == KD - 1),
                                )
                                if k < KD - 1:
                                    yield
                            nc.vector.tensor_copy(Qt[ni][:, c, :], ps[:])
                            yield

            def pump(gen, n):
                if gen is None:
                    return
                for _ in range(n):
                    if next(gen, "done") == "done":
                        return

            def drain(gen):
                if gen is not None:
                    for _ in gen:
                        pass


            ots = {}

            def emit_epilogue(qi, hp, po, Ot):
                # per-head-pair epilogue: copy both denominator rows into one
                # [1, 2*QT] SBUF tile (custom-DVE ops read garbage from PSUM
                # on HW, so bounce through SBUF), fast reciprocals (18-bit
                # accurate, far below bf16 noise), partition_broadcast,
                # then normalize each head straight out of PSUM (one SBUF
                # input only, so the DVE base-match rule doesn't apply).
                # Latency-ordered: both den copies first, then per-par
                # recip/broadcast/mul so bcast0+mul0 pipeline against
                # rc1/bcast1 instead of serializing the whole chain.
                # partition_broadcast must start at partition 0 on HW
                # (base-64 output slices produce garbage), so broadcast all
                # 128 partitions and slice both mul inputs at the same base.
                den = rcp.tile([1, 2 * QT], fp32, tag="den", name=f"den_{qi}_{hp}")
                for par in range(2):
                    nc.vector.tensor_copy(
                        den[:, par * QT : (par + 1) * QT], po[par][DH : DH + 1, :]
                    )
                rc = rcp.tile([1, 2 * QT], fp32, tag="rc", name=f"rc_{qi}_{hp}")
                rbc = rcp.tile([128, 2 * QT], fp32, tag="rbc", name=f"rbc_{qi}_{hp}")
                for par in range(2):
                    nc.vector.reciprocal_approx_fast(
                        rc[:, par * QT : (par + 1) * QT],
                        den[:, par * QT : (par + 1) * QT],
                    )
                for par in range(2):
                    nc.gpsimd.partition_broadcast(
                        rbc[:, par * QT : (par + 1) * QT],
                        rc[:, par * QT : (par + 1) * QT],
                    )
                    lo, hi = par * DH, (par + 1) * DH
                    nc.vector.tensor_mul(
                        out=Ot[hp][lo:hi, :],
                        in0=po[par][0:DH, :],
                        in1=rbc[lo:hi, par * QT : (par + 1) * QT],
                    )

            def emit_att_hp(qi, hp, fillers=None, drain_at=None, defer_epilogue=False,
                            pump_n=3, pump_every=1):
                """Attention j-loop + normalize epilogue for head pair hp.
                fillers: projection generator pumped pump_n matmuls per j step
                to fill PE gaps while ACT runs exp. Lower pump rates leave a
                bigger end-of-stretch drain burst, which is what covers the
                epilogue's DVE/GpSimd chain at the block boundary. drain_at:
                j index at which the filler generator must be fully drained
                (its outputs are needed by that j's own matmuls).
                defer_epilogue: emit the j-loop only and return the po tiles;
                the caller emits the epilogue later (so a projection drain
                isn't stuck behind the epilogue's DVE chain)."""
                if hp == 0:
                    # per-chunk Ot tiles: the output projection's c-th matmul
                    # only depends on head pair c
                    ots[qi] = [
                        otp.tile([128, QT], bf16, tag=f"ot{c}", name=f"ot_{qi}_{c}")
                        for c in range(IC)
                    ]
                Ot = ots[qi]
                njc = (qi + 1) * RB
                po = [
                    opp.tile([DH + 1, QT], fp32, tag="o", name=f"po_{qi}_{hp}_{p}")
                    for p in range(2)
                ]
                def emit_o(j, e, jb, jr, q0):
                    # O matmuls for step j, accumulated with the softmax
                    # denominator via the ones-augmented V
                    for par in range(2):
                        h = 2 * hp + par
                        nc.tensor.matmul(
                            po[par][:, q0:QT],
                            Vaug[jb][:, jr, h, :],
                            e[:, par * QT + q0 : (par + 1) * QT],
                            start=(j == 0),
                            stop=(j == njc - 1),
                            skip_group_check=True,
                        )

                pending_o = None
                for j in range(njc):
                    if drain_at is not None and j == drain_at:
                        drain(fillers)
                    elif j % pump_every == 0:
                        pump(fillers, pump_n)
                    # both heads of the pair share one [128, 1024] PSUM
                    # (2 banks) -> a single exp per j
                    ps = stp.tile([128, 2 * QT], fp32, tag="st", name=f"st_{qi}_{hp}_{j}")
                    jb, jr = j // RB, j % RB
                    if j < qi * RB:
                        # off-diagonal chunk: full query range
                        q0 = 0
                        for par in range(2):
                            lo, hi = par * DH, (par + 1) * DH
                            nc.tensor.matmul(
                                ps[:, par * QT : (par + 1) * QT],
                                Kt[jb][lo:hi, hp, jr * KC : (jr + 1) * KC],
                                Qt[qi][lo:hi, hp, :],
                                start=True,
                                stop=True,
                            )
                        e = ep.tile([128, 2 * QT], bf16, tag="e", name=f"e_{qi}_{hp}_{j}")
                        nc.scalar.activation(e[:], ps[:], Exp)
                    else:
                        # diagonal chunk (keys r*128..r*128+127 within the
                        # query block): query columns < r*128 are fully
                        # causal-masked -> skip them entirely in the S
                        # matmul, the exp, and the O matmul. The causal
                        # boundary is a triangle inside the [*, 128] block
                        # at columns r*128..r*128+127 (keep where c-p >= 0).
                        r = j - qi * RB
                        q0 = r * KC
                        for par in range(2):
                            lo, hi = par * DH, (par + 1) * DH
                            nc.tensor.matmul(
                                ps[:, par * QT + q0 : (par + 1) * QT],
                                Kt[jb][lo:hi, hp, jr * KC : (jr + 1) * KC],
                                Qt[qi][lo:hi, hp, q0:QT],
                                start=True,
                                stop=True,
                            )
                        e = ep.tile([128, 2 * QT], bf16, tag="e", name=f"e_{qi}_{hp}_{j}")
                        e3 = e[:].rearrange("p (g c) -> p g c", g=2)
                        ps3 = ps[:].rearrange("p (g c) -> p g c", g=2)
                        nc.scalar.activation(e3[:, :, q0:QT], ps3[:, :, q0:QT], Exp)
                        nc.gpsimd.affine_select(
                            out=e3[:, :, q0 : q0 + KC],
                            in_=e3[:, :, q0 : q0 + KC],
                            compare_op=mybir.AluOpType.is_ge,
                            fill=0.0,
                            base=0,
                            channel_multiplier=-1,
                            pattern=[[0, 2], [1, KC]],
                        )
                    # software-pipeline the O matmuls one j behind: by the
                    # time O(j-1) reaches the head of the PE queue its
                    # exp/affine_select chain resolved a full period ago, so
                    # the e-latency never stalls the PE (which would
                    # otherwise stall on every sparsely-filled j step)
                    if pending_o is not None:
                        emit_o(*pending_o)
                    pending_o = (j, e, jb, jr, q0)
                emit_o(*pending_o)
                if defer_epilogue:
                    return po
                emit_epilogue(qi, hp, po, Ot)
                return None

            def final_gen(qi, copies_on_dve=False, ms_list=None):
                # output projection, two waves of two 128-row chunks each,
                # c-outermost within a wave: the 12 matmuls on head pairs
                # 0..2 cover the latency of head pair 3's epilogue chain
                # (den -> recip -> broadcast -> mul on DVE/GpSimd).
                # PSUM tiles share the "st" tag (the mm pool stays
                # projections-only, so attention and projections never
                # serialize on PSUM slot order). Yields one step per matmul
                # so the driver can also pump it into a later attention
                # stretch as filler work.
                Ot = ots[qi]
                if ms_list is None:
                    ms_list = [[0, 1], [2, 3]]
                for ms in ms_list:
                    pss = {
                        m: stp.tile([128, 2 * QT], fp32, tag="st", name=f"fps_{qi}_{m}")
                        for m in ms
                    }
                    for c in range(IC):
                        for m in ms:
                            for nn in range(DIM // 512):
                                nc.tensor.matmul(
                                    pss[m][:, nn * 512 : (nn + 1) * 512],
                                    Ot[c][:, m * 128 : (m + 1) * 128],
                                    wo_sb[:, c, nn * 512 : (nn + 1) * 512],
                                    start=(c == 0),
                                    stop=(c == IC - 1),
                                )
                            yield
                    for m in ms:
                        # bounce on ACT normally (psum release must not chain
                        # through the epilogue ops in the DVE queue at a block
                        # boundary); when this final is deferred into an
                        # attention stretch, bounce on DVE instead — the
                        # epilogue chains have drained by pump time, and the
                        # ACT exp stream is pacing that stretch, so keep it
                        # unpolluted. (GpSimd can't read PSUM; DMA can't
                        # either.) The last wave is split in half so the
                        # output DMA starts a copy earlier.
                        ost = osp.tile([128, DIM], fp32, tag="ost", name=f"ost_{qi}_{m}")
                        copy = nc.vector.tensor_copy if copies_on_dve else nc.scalar.copy
                        last = qi == NQB - 1 and m == max(max(g) for g in ms_list)
                        for h0, h1 in ([(0, DIM // 2), (DIM // 2, DIM)] if last else [(0, DIM)]):
                            copy(ost[:, h0:h1], pss[m][:, h0:h1])
                            nc.sync.dma_start(
                                out[qi * QT + m * 128 : qi * QT + (m + 1) * 128, h0:h1],
                                ost[:, h0:h1],
                            )
                    yield

            def emit_final(qi):
                drain(final_gen(qi))

            def deferred_final_gen(qi, ms=None):
                # output projection variant for finals deferred into a later
                # attention stretch: psum comes from the mm pool (idle once
                # the projections are done) so the "st" ring stays purely
                # attention tiles — interleaving foreign allocations there
                # costs the S/exp stream its pipeline depth. Per-m pairs of
                # [128, 512] groups, one output row-chunk at a time.
                Ot = ots[qi]
                for m in (range(QT // 128) if ms is None else ms):
                    pss = [
                        mmp.tile([128, 512], fp32, tag="mm", name=f"dfps_{qi}_{m}_{nn}")
                        for nn in range(DIM // 512)
                    ]
                    for c in range(IC):
                        for nn in range(DIM // 512):
                            nc.tensor.matmul(
                                pss[nn][:],
                                Ot[c][:, m * 128 : (m + 1) * 128],
                                wo_sb[:, c, nn * 512 : (nn + 1) * 512],
                                start=(c == 0),
                                stop=(c == IC - 1),
                            )
                        yield
                    ost = osp.tile([128, DIM], fp32, tag="ost", name=f"dost_{qi}_{m}")
                    for nn in range(DIM // 512):
                        nc.vector.tensor_copy(
                            ost[:, nn * 512 : (nn + 1) * 512], pss[nn][:]
                        )
                        nc.sync.dma_start(
                            out[
                                qi * QT + m * 128 : qi * QT + (m + 1) * 128,
                                nn * 512 : (nn + 1) * 512,
                            ],
                            ost[:, nn * 512 : (nn + 1) * 512],
                        )
                    yield

            # Software-pipelined emission with one-block skew, pumping
            # projection matmuls into the PE gaps of the ACT-paced attention
            # j-loops. Pump rates are set so each generator is mostly but
            # not fully consumed in the loop: the leftover drain burst is
            # the PE work that covers the last head pair's epilogue chain
            # at the block boundary (emitted after the drain so the drain's
            # matmuls aren't queued behind the epilogue's DVE chain).
            # Block 3's K projection is deferred into the final attention
            # stretch (its keys are only needed from j=12 on), and block 2's
            # output projection is deferred there too (the only stretch with
            # no projection work left to pump).
            drain(proj_gen(0, "kqv"))
            emit_x_loads(1)
            g1 = proj_gen(1, "qkv")
            for hp in range(HPC // 2 - 1):
                emit_att_hp(0, hp, fillers=g1)
            po3 = emit_att_hp(0, 3, fillers=g1, defer_epilogue=True)
            drain(g1)
            emit_epilogue(0, 3, po3, ots[0])
            emit_final(0)
            emit_x_loads(2)
            g2 = proj_gen(2, "qkv")
            for hp in range(HPC // 2 - 1):
                emit_att_hp(1, hp, fillers=g2, pump_n=2)
            po3 = emit_att_hp(1, 3, fillers=g2, defer_epilogue=True, pump_n=2)
            drain(g2)
            emit_epilogue(1, 3, po3, ots[1])
            emit_x_loads(3)
            g3 = proj_gen(3, "qv")
            for hp in range(HPC // 2 - 1):
                emit_att_hp(2, hp, fillers=g3, pump_n=1)
            po3 = emit_att_hp(2, 3, fillers=g3, defer_epilogue=True, pump_n=1)
            drain(g3)
            emit_epilogue(2, 3, po3, ots[2])
            g3k = proj_gen(3, "k")
            emit_att_hp(3, 0, fillers=g3k, drain_at=3 * RB)
            # tail fillers for the last (otherwise unfilled, exp-paced)
            # attention stretch: block 1's and block 2's output projections,
            # both deferred here because this is the only stretch whose exp
            # work exceeds its own PE work (needs ~9us of outside PE filler;
            # otp bufs=3 keeps Ot(1) alive until its deferred consumer).
            # final(3) can NOT be pre-pumped here: its wave-A psum tiles
            # would sit open across head pair 3's j-loop while sharing the
            # "st" ring with the attention tiles -> ring deadlock.
            f1 = deferred_final_gen(1)
            f2 = deferred_final_gen(2)

            def tail_fillers():
                yield from f1
                yield from f2

            tg = tail_fillers()
            emit_att_hp(3, 1, fillers=tg, pump_n=1)
            emit_att_hp(3, 2, fillers=tg, pump_n=1)
            # throttle the pump in the last j-loop so ~6 filler yields
            # survive it: drained right after the last epilogue is emitted,
            # they are the PE work that covers its den->recip->broadcast->mul
            # chain (the 12 head-pair-0..2 matmuls of final(3) alone are
            # ~2.5us of cover against a ~5us chain)
            po3 = emit_att_hp(3, 3, fillers=tg, defer_epilogue=True, pump_n=1,
                              pump_every=8)
            emit_epilogue(3, 3, po3, ots[3])
            drain(tg)
            # final(3): half through the stp pool (ACT copies), half through
            # the now-idle mm pool (DVE copies), emitted interleaved so the
            # two psum rings and the two copy engines work in parallel
            # instead of wave B serializing behind wave A's copies
            _end = object()
            fa = final_gen(3, ms_list=[[0, 1]])
            fb = deferred_final_gen(3, ms=[2, 3])
            while fa is not None or fb is not None:
                if fa is not None and next(fa, _end) is _end:
                    fa = None
                if fb is not None and next(fb, _end) is _end:
                    fb = None
    nc.compile()
    return nc


def _get_program():
    if "nc" not in _cache:
        _cache["nc"] = _build_program()
    return _cache["nc"]


def _make_in_maps(x, W_qkv, W_out):
    import ml_dtypes

    bf16 = ml_dtypes.bfloat16
    scale = DH ** -0.5
    xTs = [np.ascontiguousarray(x[b].T).astype(bf16) for b in range(B)]
    in_maps = []
    for core in range(8):
        b, g = core // 2, core % 2
        cols = slice(g * INNER, (g + 1) * INNER)
        in_maps.append(
            {
                "xT": xTs[b],
                "wq": (np.ascontiguousarray(W_qkv[:, cols]) * np.float32(scale)).astype(bf16),
                "wk": np.ascontiguousarray(W_qkv[:, 1024:][:, cols]).astype(bf16),
                "wv": np.ascontiguousarray(W_qkv[:, 2048:][:, cols]).astype(bf16),
                "wo": np.ascontiguousarray(W_out[g * INNER : (g + 1) * INNER, :]).astype(bf16),
            }
        )
    return in_maps


def _run(inputs, trace=False, trace_cores=None, tmpdir=None):
    from concourse.bass_utils import run_bass_kernel_spmd

    x = np.asarray(inputs["x"], dtype=np.float32)
    W_qkv = np.asarray(inputs["W_qkv"], dtype=np.float32)
    W_out = np.asarray(inputs["W_out"], dtype=np.float32)
    b_out = np.asarray(inputs["b_out"], dtype=np.float32)

    nc = _get_program()
    in_maps = _make_in_maps(x, W_qkv, W_out)
    res = run_bass_kernel_spmd(
        nc,
        in_maps,
        core_ids=list(range(8)),
        trace=trace,
        trace_cores=trace_cores,
        tmpdir=tmpdir,
    )
    outp = np.empty((B, N, DIM), dtype=np.float32)
    for b in range(B):
        outp[b] = res.results[2 * b]["out"] + res.results[2 * b + 1]["out"] + b_out
    return outp, res


def kernel(**inputs):
    outp, _ = _run(inputs, trace=False)
    return outp
